# revision 1
# baseline (speedup 1.0000x reference)
"""Trainium2 Bass kernel for nn_AttEncode: 6-layer weight-shared encoder.

Strategy: data-parallel over batch (B=32 -> 4 per core x 8 cores), zero
collectives. Per-core state kept in transposed layout hT=[D(part), S(free)]
so every matmul's contraction dim lands on partitions. The H-way `tile`
concat + fuse_w collapses to a single 200x200 matmul with
fuse_sum = fuse_w.reshape(8,200,200).sum(0). Softmax: scores are symmetric
(q==k==v), max|score/sqrt(D)| ~= 56 so exp() is safe in fp32 without
max-subtraction; E = exp(s/sqrt(D)) is then exactly symmetric, so the
attention matmul consumes the stored E directly as its [k, q] operand. The
row-sum normalizer comes for free from a ones-column appended to the
attention lhsT; biases ride in augmented weight rows. All matmuls run in
float32r (1 cycle/row at free-dim >= 256).
"""

import os
import numpy as np
from contextlib import ExitStack

import concourse.bass as bass
import concourse.tile as tile
from concourse import bacc, mybir
from concourse import bass_utils
from concourse.masks import make_identity
from concourse.bass import ts

B, S, D, H, STACK, V = 32, 1024, 200, 8, 6, 32000
N_CORES = 8
NB = B // N_CORES          # batches per core
NST = S // 128             # seq tiles of 128
D0, D1 = 128, D - 128      # d-dim partition split 128 + 72
SCALE = 1.0 / float(np.sqrt(np.float32(D)))
INV_N = 1.0 / float(S * D)

F32 = mybir.dt.float32
F32R = mybir.dt.float32r
I32 = mybir.dt.int32
AF = mybir.ActivationFunctionType
ALU = mybir.AluOpType


def _r(ap):
    return ap.bitcast(F32R)


def build_nc(nb=NB, stack=STACK, io_reps=1):
    nc = bacc.Bacc("TRN2", target_bir_lowering=False, debug=False,
                   enable_asserts=False)

    x_d = nc.dram_tensor("x", [nb, S], I32, kind="ExternalInput").ap()
    embed_d = nc.dram_tensor("embed", [V, D], F32, kind="ExternalInput").ap()
    pos_d = nc.dram_tensor("pos", [S, D], F32, kind="ExternalInput").ap()
    wq_d = nc.dram_tensor("wq", [D, D], F32R, kind="ExternalInput").ap()
    wqp0_d = nc.dram_tensor("wqp0", [128, 256], F32R, kind="ExternalInput").ap()
    wqp1_d = nc.dram_tensor("wqp1", [73, 256], F32R, kind="ExternalInput").ap()
    fs0_d = nc.dram_tensor("fs0", [128, D], F32R, kind="ExternalInput").ap()
    fs1_d = nc.dram_tensor("fs1", [73, D], F32R, kind="ExternalInput").ap()
    w1_d = nc.dram_tensor("w1a", [D, D + 1], F32R, kind="ExternalInput").ap()
    w2a0_d = nc.dram_tensor("w2a0", [128, D], F32R, kind="ExternalInput").ap()
    w2a1_d = nc.dram_tensor("w2a1", [73, D], F32R, kind="ExternalInput").ap()
    qb_d = nc.dram_tensor("qb", [D, 1], F32, kind="ExternalInput").ap()
    b1a_d = nc.dram_tensor("b1a", [D + 1, 1], F32, kind="ExternalInput").ap()
    cones_d = nc.dram_tensor("cones", [1, S], F32R, kind="ExternalInput").ap()
    ident_d = nc.dram_tensor("ident", [128, 128], F32R, kind="ExternalInput").ap()
    y_d = nc.dram_tensor("y", [nb, S, D], F32, kind="ExternalOutput").ap()

    with tile.TileContext(nc) as tc, ExitStack() as ctx:
        const = ctx.enter_context(tc.tile_pool(name="const", bufs=1))
        state = ctx.enter_context(tc.tile_pool(name="state", bufs=1))
        p_qkvT = ctx.enter_context(tc.tile_pool(name="p_qkvT", bufs=2))
        p_E = ctx.enter_context(tc.tile_pool(name="p_E", bufs=8))
        p_qkvn = ctx.enter_context(tc.tile_pool(name="p_qkvn", bufs=2))
        p_cT = ctx.enter_context(tc.tile_pool(name="p_cT", bufs=2))
        p_f1 = ctx.enter_context(tc.tile_pool(name="p_f1", bufs=2))
        p_u = ctx.enter_context(tc.tile_pool(name="p_u", bufs=1))
        p_scr = ctx.enter_context(tc.tile_pool(name="p_scr", bufs=1))
        p_row = ctx.enter_context(tc.tile_pool(name="p_row", bufs=2))
        p_sm = ctx.enter_context(tc.tile_pool(name="p_sm", bufs=4))
        p_emb = ctx.enter_context(tc.tile_pool(name="p_emb", bufs=3))
        ps_big = ctx.enter_context(tc.tile_pool(name="ps_big", bufs=3, space="PSUM"))
        ps_sm = ctx.enter_context(tc.tile_pool(name="ps_sm", bufs=2, space="PSUM"))

        # ---- constants & weights ----
        id128 = const.tile([128, 128], F32, tag="id128")
        make_identity(nc, id128[:])
        ones_col = const.tile([128, 1], F32R, tag="ones_col")
        nc.sync.dma_start(ones_col[:], cones_d[0, 0:128].rearrange("(p one) -> p one", one=1))
        ones_row = const.tile([1, 128], F32R, tag="ones_row")
        nc.sync.dma_start(ones_row[:], cones_d[0:1, 0:128])
        identr = const.tile([128, 128], F32R, tag="identr")
        nc.sync.dma_start(identr[:], ident_d[:])
        eps_ap = const.tile([1, 1], F32, tag="eps")
        nc.vector.memset(eps_ap[:], 1e-5)

        wq0 = const.tile([128, D], F32R, tag="wq0")
        nc.sync.dma_start(wq0[:], wq_d[0:128, :])
        wq1 = const.tile([D1, D], F32R, tag="wq1")
        nc.sync.dma_start(wq1[:], wq_d[128:D, :])
        wqp0 = const.tile([128, 256], F32R, tag="wqp0")
        nc.sync.dma_start(wqp0[:], wqp0_d[:])
        wqp1 = const.tile([73, 256], F32R, tag="wqp1")
        nc.sync.dma_start(wqp1[:], wqp1_d[:])
        fs0 = const.tile([128, D], F32R, tag="fs0")
        nc.sync.dma_start(fs0[:], fs0_d[:])
        fs1 = const.tile([73, D], F32R, tag="fs1")
        nc.sync.dma_start(fs1[:], fs1_d[:])
        w10 = const.tile([128, D + 1], F32R, tag="w10")
        nc.sync.dma_start(w10[:], w1_d[0:128, :])
        w11 = const.tile([D1, D + 1], F32R, tag="w11")
        nc.sync.dma_start(w11[:], w1_d[128:D, :])
        w2a0 = const.tile([128, D], F32R, tag="w2a0")
        nc.sync.dma_start(w2a0[:], w2a0_d[:])
        w2a1 = const.tile([73, D], F32R, tag="w2a1")
        nc.sync.dma_start(w2a1[:], w2a1_d[:])
        qb0 = const.tile([128, 1], F32, tag="qb0")
        nc.sync.dma_start(qb0[:], qb_d[0:128, :])
        qb1 = const.tile([D1, 1], F32, tag="qb1")
        nc.sync.dma_start(qb1[:], qb_d[128:D, :])
        b1a0 = const.tile([128, 1], F32, tag="b1a0")
        nc.sync.dma_start(b1a0[:], b1a_d[0:128, :])
        b1a1 = const.tile([73, 1], F32, tag="b1a1")
        nc.sync.dma_start(b1a1[:], b1a_d[128:D + 1, :])

        pos_t = []
        for st in range(NST):
            pt = const.tile([128, D], F32, tag=f"pos{st}")
            nc.sync.dma_start(pt[:], pos_d[ts(st, 128), :])
            pos_t.append(pt)

        # ---- per-batch persistent state: hT0 [128,1024], hT1 [73,1024] ----
        hT0 = []
        hT1 = []
        for b in range(nb):
            t0 = state.tile([128, S], F32, tag=f"hT0_{b}")
            t1 = state.tile([73, S], F32, tag=f"hT1_{b}")
            nc.sync.dma_start(_r(t1[72:73, :]), cones_d[0:1, :])  # ones row (qkv bias fold)
            hT0.append(t0)
            hT1.append(t1)

        # ---- embedding gather + pos + transpose into hT ----
        for _rep in range(io_reps):
          for b in range(nb):
            idx = p_emb.tile([128, NST], I32, tag="idx")
            nc.sync.dma_start(idx[:], x_d[b].rearrange("(t p) -> p t", p=128))
            for st in range(NST):
                g = p_emb.tile([128, D], F32, tag="g")
                nc.gpsimd.indirect_dma_start(
                    out=g[:], out_offset=None, in_=embed_d[:],
                    in_offset=bass.IndirectOffsetOnAxis(ap=idx[:, st:st + 1], axis=0),
                )
                h0 = p_emb.tile([128, D], F32, tag="h0")
                nc.vector.tensor_add(h0[:], g[:], pos_t[st][:])
                tr0 = ps_sm.tile([128, 128], F32, tag="sm")
                nc.tensor.transpose(tr0[:], h0[:, 0:128], id128[:])
                nc.scalar.copy(_r(hT0[b][:, ts(st, 128)]), tr0[:])
                tr1 = ps_sm.tile([D1, 128], F32, tag="sm")
                nc.tensor.transpose(tr1[:], h0[:, 128:D], id128[:])
                nc.scalar.copy(_r(hT1[b][0:D1, ts(st, 128)]), tr1[:])

        # ---- layers ----
        for _l in range(stack):
            for b in range(nb):
                h0t, h1t = hT0[b], hT1[b]

                # qkvT = Wq.T @ hT + qkv_b   -> [d_out(200), s] two tiles
                qkvT0 = p_qkvT.tile([128, S], F32, tag="qkvT0")
                qkvT1 = p_qkvT.tile([D1, S], F32, tag="qkvT1")
                for (mt, msl, qb_, wslice) in (
                    (qkvT0, 128, qb0, slice(0, 128)),
                    (qkvT1, D1, qb1, slice(128, D)),
                ):
                    ps = ps_big.tile([msl, S], F32, tag="big")
                    for n2 in range(2):
                        nsl = ts(n2, 512)
                        nc.tensor.matmul(ps[:, nsl], _r(wq0[:, wslice]),
                                         _r(h0t[:, nsl]), start=True, stop=False)
                        nc.tensor.matmul(ps[:, nsl], _r(wq1[:, wslice]),
                                         _r(h1t[0:D1, nsl]), start=False, stop=True)
                    nc.scalar.activation(_r(mt[:]), ps[:], AF.Identity,
                                         bias=qb_[:], scale=1.0)

                # qkv normal (padded to 256, col 200 = ones) [s, 256] x8 tiles
                qkvn = []
                for g2 in range(2):
                    ps = ps_big.tile([128, S], F32, tag="big")
                    for q in range(4):
                        st = g2 * 4 + q
                        reg = slice(q * 256, (q + 1) * 256)
                        nc.tensor.matmul(ps[:, reg], _r(h0t[:, ts(st, 128)]),
                                         _r(wqp0[:]), start=True, stop=False)
                        nc.tensor.matmul(ps[:, reg], _r(h1t[0:73, ts(st, 128)]),
                                         _r(wqp1[:]), start=False, stop=True)
                    sb = p_qkvn.tile([128, S], F32, tag="qkvn")
                    nc.vector.tensor_copy(_r(sb[:]), ps[:])
                    qkvn.append(sb)

                # scores + exp, per q-tile: E[qt] = exp(scale * qkvT.T qkvT)
                # exp's accum_out gives rowsum per q-tile; PE-transpose the
                # eight [128,1] columns into the [1, S] rowsum row ps_rs.
                E = []
                ps_rs = ps_big.tile([1, S], F32, tag="big")
                for qt in range(NST):
                    ps = ps_big.tile([128, S], F32, tag="big")
                    for n2 in range(2):
                        nsl = ts(n2, 512)
                        nc.tensor.matmul(ps[:, nsl], _r(qkvT0[:, ts(qt, 128)]),
                                         _r(qkvT0[:, nsl]), start=True, stop=False)
                        nc.tensor.matmul(ps[:, nsl], _r(qkvT1[0:D1, ts(qt, 128)]),
                                         _r(qkvT1[0:D1, nsl]), start=False, stop=True)
                    e = p_E.tile([128, S], F32, tag="E")
                    rsc = p_sm.tile([128, 1], F32, tag="rsc")
                    nc.scalar.activation(_r(e[:]), ps[:], AF.Exp, bias=0.0, scale=SCALE,
                                         accum_out=rsc[:])
                    nc.tensor.transpose(ps_rs[:, ts(qt, 128)], rsc[:], id128[:])
                    E.append(e)

                # attention out (transposed, unnormalized) + rowsum row:
                # cT_un[d,q] = sum_k qkv[k,d] * E[k,q];  lhsT = qkv cols 0:201
                # (col 200 = ones -> row 72 of tile1 = rowsum[q])
                ps_c0 = ps_big.tile([128, S], F32, tag="big")
                ps_c1 = ps_big.tile([73, S], F32, tag="big")
                for kt in range(NST):
                    base = (kt % 4) * 256
                    lsrc = qkvn[kt // 4]
                    for (psc, csl) in ((ps_c0, slice(base, base + 128)),
                                       (ps_c1, slice(base + 128, base + 201))):
                        for n2 in range(2):
                            nsl = ts(n2, 512)
                            nc.tensor.matmul(psc[:, nsl], _r(lsrc[:, csl]),
                                             _r(E[kt][:, nsl]),
                                             start=(kt == 0), stop=(kt == NST - 1))

                # B = broadcast(1/rowsum) over partitions via ones-matmul
                rr = p_row.tile([1, S], F32, tag="rr")
                with nc.allow_low_precision(reason="fp32r rounding of 1/rowsum"):
                    nc.vector.reciprocal(_r(rr[:]), ps_rs[:])
                ps_B = ps_big.tile([128, S], F32, tag="big")
                for n2 in range(2):
                    nsl = ts(n2, 512)
                    nc.tensor.matmul(ps_B[:, nsl], _r(ones_row[:]), _r(rr[:, nsl]),
                                     start=True, stop=True)

                # cT normalized; row 72 of cT1 = rowsum * (1/rowsum) ~ 1.0
                B_sb = p_cT.tile([128, S], F32, tag="Bsb")
                nc.scalar.copy(B_sb[:], ps_B[:])
                cT0 = p_cT.tile([128, S], F32, tag="cT0")
                nc.vector.tensor_mul(_r(cT0[:]), ps_c0[:], B_sb[:])
                cT1 = p_cT.tile([73, S], F32, tag="cT1")
                nc.vector.tensor_mul(_r(cT1[:]), ps_c1[:], B_sb[0:73, :])

                # fuse: tT = fuse_sum_aug.T @ cT (+fuse_b via ones row)
                # then u = tT + hT (residual) with row-sum accumulation
                ps_t0 = ps_big.tile([128, S], F32, tag="big")
                ps_t1 = ps_big.tile([D1, S], F32, tag="big")
                for (pst, msz, wsl) in ((ps_t0, 128, slice(0, 128)),
                                        (ps_t1, D1, slice(128, D))):
                    hsrc = h0t if msz == 128 else h1t
                    for n2 in range(2):
                        nsl = ts(n2, 512)
                        nc.tensor.matmul(pst[:, nsl], _r(identr[0:msz, 0:msz]),
                                         _r(hsrc[0:msz, nsl]), start=True, stop=False)
                        nc.tensor.matmul(pst[:, nsl], _r(fs0[:, wsl]),
                                         _r(cT0[:, nsl]), start=False, stop=False)
                        nc.tensor.matmul(pst[:, nsl], _r(fs1[:, wsl]),
                                         _r(cT1[:, nsl]), start=False, stop=True)
                self_ln(nc, tc, p_u, p_scr, p_sm, ps_sm, ones_col, ones_row,
                        eps_ap, ps_t0, ps_t1, h0t, h1t)

                # FFN: f1T = relu(w1a.T @ hT + b1a)  (row 72 of f1 tile1 = 1.0)
                ps_f0 = ps_big.tile([128, S], F32, tag="big")
                ps_f1 = ps_big.tile([73, S], F32, tag="big")
                for (psf, wsl) in ((ps_f0, slice(0, 128)), (ps_f1, slice(128, D + 1))):
                    for n2 in range(2):
                        nsl = ts(n2, 512)
                        nc.tensor.matmul(psf[:, nsl], _r(w10[:, wsl]),
                                         _r(h0t[:, nsl]), start=True, stop=False)
                        nc.tensor.matmul(psf[:, nsl], _r(w11[:, wsl]),
                                         _r(h1t[0:D1, nsl]), start=False, stop=True)
                f10 = p_f1.tile([128, S], F32, tag="f10")
                nc.scalar.activation(_r(f10[:]), ps_f0[:], AF.Relu, bias=b1a0[:], scale=1.0)
                f11 = p_f1.tile([73, S], F32, tag="f11")
                nc.scalar.activation(_r(f11[:]), ps_f1[:], AF.Relu, bias=b1a1[:], scale=1.0)

                # f2T = w2a.T @ f1 (+b2 via ones row), residual + LN
                ps_g0 = ps_big.tile([128, S], F32, tag="big")
                ps_g1 = ps_big.tile([D1, S], F32, tag="big")
                for (psg, msz, wsl) in ((ps_g0, 128, slice(0, 128)),
                                        (ps_g1, D1, slice(128, D))):
                    hsrc = h0t if msz == 128 else h1t
                    for n2 in range(2):
                        nsl = ts(n2, 512)
                        nc.tensor.matmul(psg[:, nsl], _r(identr[0:msz, 0:msz]),
                                         _r(hsrc[0:msz, nsl]), start=True, stop=False)
                        nc.tensor.matmul(psg[:, nsl], _r(w2a0[:, wsl]),
                                         _r(f10[:, nsl]), start=False, stop=False)
                        nc.tensor.matmul(psg[:, nsl], _r(w2a1[:, wsl]),
                                         _r(f11[:, nsl]), start=False, stop=True)
                self_ln(nc, tc, p_u, p_scr, p_sm, ps_sm, ones_col, ones_row,
                        eps_ap, ps_g0, ps_g1, h0t, h1t)

        # ---- output: transpose hT back to [s, d] and DMA out ----
        for _rep in range(io_reps):
          for b in range(nb):
            for st in range(NST):
                o = p_emb.tile([128, D], F32, tag="o")
                tr0 = ps_sm.tile([128, 128], F32, tag="sm")
                nc.tensor.transpose(tr0[:], hT0[b][:, ts(st, 128)], id128[:])
                nc.scalar.copy(o[:, 0:128], tr0[:])
                tr1 = ps_sm.tile([128, D1], F32, tag="sm")
                nc.tensor.transpose(tr1[:], hT1[b][0:D1, ts(st, 128)],
                                    id128[0:D1, 0:D1])
                nc.scalar.copy(o[:, 128:D], tr1[:])
                nc.sync.dma_start(y_d[b, ts(st, 128), :], o[:])

    nc.compile()
    return nc


def self_ln(nc, tc, p_u, p_scr, p_sm, ps_sm, ones_col, ones_row,
            eps_ap, ps0, ps1, h0t, h1t):
    """u = ps + hT; joint layernorm over all (S, D); writes back into hT."""
    with nc.allow_low_precision(reason="fp32r rounding of LN partial sums"):
        pq0 = p_sm.tile([128, 2], F32, tag="pq0")
        pq1 = p_sm.tile([D1, 2], F32, tag="pq1")
        nc.vector.tensor_reduce(out=_r(pq0[:, 0:1]), in_=ps0[:],
                                axis=mybir.AxisListType.X, op=ALU.add)
        nc.vector.tensor_reduce(out=_r(pq1[:, 0:1]), in_=ps1[:],
                                axis=mybir.AxisListType.X, op=ALU.add)
        q_t0 = p_sm.tile([128, 1], F32, tag="q_t0")
        scr0 = p_scr.tile([128, S], F32, tag="scr")
        nc.scalar.activation(scr0[:], ps0[:], AF.Square, accum_out=_r(q_t0[:]))
        q_t1 = p_sm.tile([D1, 1], F32, tag="q_t1")
        scr1 = p_scr.tile([D1, S], F32, tag="scr")
        nc.scalar.activation(scr1[:], ps1[:], AF.Square, accum_out=_r(q_t1[:]))
        nc.vector.tensor_copy(_r(pq0[:, 1:2]), q_t0[:])
        nc.vector.tensor_copy(_r(pq1[:, 1:2]), q_t1[:])

    # cross-partition totals: [1,2] = ones.T @ [sum|sumsq]  (N=2: fp32r needs even N)
    stp = ps_sm.tile([1, 2], F32, tag="sm")
    nc.tensor.matmul(stp[:], _r(ones_col[:]), _r(pq0[:]), start=True, stop=False)
    nc.tensor.matmul(stp[:], _r(ones_col[0:D1, :]), _r(pq1[:]), start=False, stop=True)

    stat = p_sm.tile([1, 2], F32, tag="stat")     # [mean, meansq]
    nc.scalar.mul(stat[:], stp[:], INV_N)
    msq = p_sm.tile([1, 1], F32, tag="msq")
    nc.scalar.activation(msq[:], stat[:, 0:1], AF.Square)
    var = p_sm.tile([1, 1], F32, tag="var")
    nc.vector.tensor_sub(var[:], stat[:, 1:2], msq[:])
    std = p_sm.tile([1, 1], F32, tag="std")
    nc.scalar.activation(std[:], var[:], AF.Sqrt, bias=eps_ap[:], scale=1.0)
    pk = p_sm.tile([1, 2], F32, tag="pk")         # [rstd, mean*rstd]
    with nc.allow_low_precision(reason="fp32r rounding of rstd"):
        nc.vector.reciprocal(_r(pk[:, 0:1]), std[:])
    nc.vector.tensor_mul(_r(pk[:, 1:2]), stat[:, 0:1], pk[:, 0:1])

    bcp = ps_sm.tile([128, 2], F32, tag="sm")
    nc.tensor.matmul(bcp[:], _r(ones_row[:]), _r(pk[:]), start=True, stop=True)
    bc = p_sm.tile([128, 2], F32, tag="bc")
    nc.scalar.copy(bc[:], bcp[:])

    nc.vector.tensor_scalar(out=_r(h0t[:]), in0=ps0[:], scalar1=bc[:, 0:1],
                            scalar2=bc[:, 1:2], op0=ALU.mult, op1=ALU.subtract)
    nc.vector.tensor_scalar(out=_r(h1t[0:D1, :]), in0=ps1[:], scalar1=bc[0:D1, 0:1],
                            scalar2=bc[0:D1, 1:2], op0=ALU.mult, op1=ALU.subtract)


def prep_weights(inputs):
    """Host-side constant folding of the shared weights."""
    f = {}
    qkv_w = np.ascontiguousarray(inputs["qkv_w"], np.float32)
    qkv_b = np.ascontiguousarray(inputs["qkv_b"], np.float32)
    fuse_sum = inputs["fuse_w"].astype(np.float32).reshape(H, D, D).sum(0)
    fuse_b = np.ascontiguousarray(inputs["fuse_b"], np.float32)
    w1 = np.ascontiguousarray(inputs["w1"], np.float32)
    b1 = np.ascontiguousarray(inputs["b1"], np.float32)
    w2 = np.ascontiguousarray(inputs["w2"], np.float32)
    b2 = np.ascontiguousarray(inputs["b2"], np.float32)

    f["wq"] = qkv_w
    # qkv-normal weights padded to 256 cols; K-chunk1 augmented with the
    # qkv_b row; its col 200 = 1.0 emits the ones column used for rowsums.
    wqp0 = np.zeros((128, 256), np.float32); wqp0[:, :D] = qkv_w[:128]
    wqp1 = np.zeros((73, 256), np.float32)
    wqp1[:72, :D] = qkv_w[128:]; wqp1[72, :D] = qkv_b; wqp1[72, 200] = 1.0
    f["wqp0"], f["wqp1"] = wqp0, wqp1
    f["fs0"] = np.ascontiguousarray(fuse_sum[:128])
    fs1 = np.zeros((73, D), np.float32)
    fs1[:72] = fuse_sum[128:]; fs1[72] = fuse_b
    f["fs1"] = fs1
    # w1 with an extra zero output column; relu bias row 200 = 1.0 makes
    # f1's row 72 of tile1 equal relu(0 + 1) = 1.0 (the ones row for b2).
    w1a = np.zeros((D, D + 1), np.float32); w1a[:, :D] = w1
    f["w1a"] = w1a
    f["w2a0"] = np.ascontiguousarray(w2[:128])
    w2a1 = np.zeros((73, D), np.float32)
    w2a1[:72] = w2[128:]; w2a1[72] = b2
    f["w2a1"] = w2a1
    f["qb"] = qkv_b.reshape(D, 1)
    b1a = np.zeros((D + 1, 1), np.float32); b1a[:D, 0] = b1; b1a[D, 0] = 1.0
    f["b1a"] = b1a
    return f


_NC_CACHE = {}


def get_nc(nb=NB, stack=STACK):
    key = (nb, stack)
    if key not in _NC_CACHE:
        _NC_CACHE[key] = build_nc(nb, stack)
    return _NC_CACHE[key]


def make_in_maps(inputs, n_cores=N_CORES, nb=NB):
    w = prep_weights(inputs)
    x = np.asarray(inputs["x"]).astype(np.int32)
    embed = np.ascontiguousarray(inputs["embed"], np.float32)
    pos = np.ascontiguousarray(inputs["pos"], np.float32)
    in_maps = []
    for c in range(n_cores):
        m = {"x": np.ascontiguousarray(x[c * nb:(c + 1) * nb]),
             "embed": embed, "pos": pos,
             "cones": np.ones((1, S), np.float32),
             "ident": np.eye(128, dtype=np.float32)}
        m.update(w)
        in_maps.append(m)
    return in_maps


def kernel(**inputs) -> np.ndarray:
    nc = get_nc()
    in_maps = make_in_maps(inputs)
    res = bass_utils.run_bass_kernel_spmd(nc, in_maps, core_ids=list(range(N_CORES)))
    out = np.concatenate([res.results[c]["y"] for c in range(N_CORES)], axis=0)
    return out.astype(np.float32)

